# revision 1
# baseline (speedup 1.0000x reference)
"""Causal GQA self-attention (B=1, T=2048, C=2048, 32 heads / 8 KV groups,
head_size 64, partial RoPE 16) on 8 Trainium2 NeuronCores.

Sharding: tensor-parallel over the 8 query groups. Core g computes its
group's qkv projection (x @ W_attn_g.T, feature-major), RoPE, causal
attention for its 4 heads, then an AllToAll redistributes the attention
output so core g holds y[:, t_slice_g] for ALL heads; each core then
computes out[t_slice_g, :] = y_slice @ W_proj.T + b_proj.

All matmuls run in float32r (TF32-like, ~2e-4 rel err, full PE rate).
DMAs are batched into ~2MB transfers (per-DMA descriptor-gen is ~0.6us
of serialized HWDGE time); qkv PSUM evacuation rides the ScalarE; the
softmax denominators are normalized in one batched pass per head pair.
"""
import sys
from contextlib import ExitStack

sys.path.insert(0, "/opt/trn_rl_repo")

import numpy as np
import concourse.bass as bass
import concourse.mybir as mybir
import concourse.tile as tile
from concourse import bacc
from concourse.bass_utils import run_bass_kernel_spmd
from concourse.masks import make_identity

F32 = mybir.dt.float32
F32R = mybir.dt.float32r
BF16 = mybir.dt.bfloat16
USE_BF16 = True   # bf16 for the big DMA streams (x, W_attn, W_proj, y)
EXP = mybir.ActivationFunctionType.Exp
IDENT = mybir.ActivationFunctionType.Identity

NCORES = 8
T = 2048
C = 2048
HS = 64
QG = 384          # per-group qkv features: 4 q heads + k + v
TS = T // NCORES  # 256, per-core output T slice
SCALE = 0.125     # 1/sqrt(64)
NKB = T // 128    # 16 key blocks
NCH = T // 512    # 4 qt chunks
NEG = -1.0e30

_nc_cache = {}


def _build(for_sim=False, reps=1):
    BIG = BF16 if USE_BF16 else F32R
    nc = bacc.Bacc("TRN2", target_bir_lowering=False, debug=False,
                   num_devices=NCORES)
    xT = nc.declare_dram_parameter("xT", [C, T], BIG, isOutput=False)
    wqkvT = nc.declare_dram_parameter("wqkvT", [C, QG], BIG, isOutput=False)
    bqkv = nc.declare_dram_parameter("bqkv", [QG, 1], F32, isOutput=False)
    ropeC = nc.declare_dram_parameter("ropeC", [128, T], F32R, isOutput=False)
    ropeS = nc.declare_dram_parameter("ropeS", [128, T], F32R, isOutput=False)
    wprojT = nc.declare_dram_parameter("wprojT", [C, C], BIG, isOutput=False)
    bproj = nc.declare_dram_parameter("bproj", [1, C], F32, isOutput=False)
    out = nc.declare_dram_parameter("out", [TS, C], F32, isOutput=True)

    with tile.TileContext(nc) as tc:
      for _rep in range(reps):
        with (
            tc.tile_pool(name="persist", bufs=1) as persist,
            tc.tile_pool(name="dram", bufs=1, space="DRAM") as dram,
        ):
            # q0|q1, q2|q3, k|v  — feature-major [feat, T]
            qkv0 = persist.tile([128, T], F32R)
            qkv1 = persist.tile([128, T], F32R)
            qkv2 = persist.tile([128, T], F32R)
            kdup = persist.tile([128, T], F32R)       # rows 64:128 = k copy
            v_sb = persist.tile([128, NKB, 65], F32R)  # v t-major + ones col
            yts = [persist.tile([64, T], BIG, name=f"yts{i}", tag=f"yts{i}")
                   for i in range(4)]
            bprojb = persist.tile([128, C], F32)
            identf = persist.tile([128, 128], F32)
            ident = persist.tile([128, 128], F32R)
            mask128 = persist.tile([128, 128], F32)
            onecol = persist.tile([128, 1], F32)

            make_identity(nc, identf[:])
            nc.vector.tensor_copy(ident[:], identf[:])
            nc.gpsimd.memset(mask128[:], 0.0)
            # mask128[p, c] = 0 if c >= p else NEG  (keep kpos <= qt)
            nc.gpsimd.affine_select(
                out=mask128[:], in_=mask128[:],
                compare_op=mybir.AluOpType.is_ge, fill=NEG,
                base=0, pattern=[[1, 128]], channel_multiplier=-1,
            )
            nc.vector.memset(onecol[:], 1.0)
            bp = bproj[0, :]
            nc.scalar.dma_start(
                bprojb[:],
                bass.AP(tensor=bp.tensor, offset=bp.offset,
                        ap=[[0, 128]] + list(bp.ap)),
            )

            # ---------------- Phase 1: qkv projection + bias + rope --------
            with (
                tc.tile_pool(name="wq", bufs=1) as wqp,
                tc.tile_pool(name="xt", bufs=4) as xtp,
                tc.tile_pool(name="rope", bufs=1) as ropep,
                tc.tile_pool(name="ps1", bufs=2, space="PSUM") as ps1,
            ):
                wq_sb = wqp.tile([128, 16 * QG], BIG)
                for wql in range(4):  # split so the first matmuls start early
                    nc.scalar.dma_start(
                        wq_sb[:, wql * 4 * QG:(wql + 1) * 4 * QG].rearrange(
                            "p (ct f) -> p ct f", ct=4),
                        wqkvT.ap()[wql * 512:(wql + 1) * 512, :].rearrange(
                            "(ct p) f -> p ct f", p=128),
                    )
                b_sb = wqp.tile([128, 3], F32)
                nc.scalar.dma_start(
                    b_sb[:].rearrange("p (i o) -> p i o", i=3),
                    bqkv.ap().rearrange("(i p) o -> p i o", p=128),
                )
                ropeC_sb = ropep.tile([128, T], F32R)
                ropeS_sb = ropep.tile([128, T], F32R)
                rtmp = ropep.tile([128, T], F32R)
                nc.scalar.dma_start(ropeC_sb[:], ropeC[:])
                nc.scalar.dma_start(ropeS_sb[:], ropeS[:])
                nc.vector.memset(rtmp[:].bitcast(F32), 0.0)

                qkv_tiles = [qkv0, qkv1, qkv2]
                for tch in range(NCH):
                    tsl = slice(tch * 512, tch * 512 + 512)
                    pq = [ps1.tile([128, 512], F32, name=f"pq{i}", tag=f"pq{i}")
                          for i in range(3)]
                    for ch in range(2):  # 2MB x-tile halves (8 c-blocks each)
                        xt = xtp.tile([128, 8, 512], BIG)
                        nc.sync.dma_start(
                            xt[:],
                            xT[ch * 1024:ch * 1024 + 1024, tsl].rearrange(
                                "(ct p) t -> p ct t", p=128),
                        )
                        for c8 in range(8):
                            ct = ch * 8 + c8
                            for fi in range(3):
                                nc.tensor.matmul(
                                    pq[fi][:],
                                    wq_sb[:, ct * QG + fi * 128:ct * QG + fi * 128 + 128],
                                    xt[:, c8, :],
                                    start=(ct == 0), stop=(ct == 15),
                                )
                    for fi in (2, 0, 1):  # k/v first: both pairs' m1 need k
                        # evac + per-partition bias on ScalarE
                        nc.scalar.activation(
                            qkv_tiles[fi][:, tsl], pq[fi][:], IDENT,
                            bias=b_sb[:, fi:fi + 1],
                        )

                    # per-chunk rope (lets attention on chunk j start as soon
                    # as chunks <= j are done). C has 1.0 / S has 0.0 on
                    # non-rope rows, so full-width ops are identity there
                    # (q tiles). k tile only touches rows 0:16 (v at 64:128
                    # must not change).
                    for ti, full in ((qkv2, False), (qkv0, True), (qkv1, True)):
                        nc.gpsimd.dma_start(rtmp[0:8, tsl], ti[8:16, tsl])
                        nc.gpsimd.dma_start(rtmp[8:16, tsl], ti[0:8, tsl])
                        if full:
                            nc.gpsimd.dma_start(rtmp[64:72, tsl], ti[72:80, tsl])
                            nc.gpsimd.dma_start(rtmp[72:80, tsl], ti[64:72, tsl])
                            r = slice(0, 128)
                        else:
                            r = slice(0, 16)
                        nc.vector.tensor_mul(rtmp[r, tsl], rtmp[r, tsl],
                                             ropeS_sb[r, tsl])
                        nc.vector.tensor_mul(ti[r, tsl], ti[r, tsl],
                                             ropeC_sb[r, tsl])
                        nc.vector.tensor_add(ti[r, tsl], ti[r, tsl],
                                             rtmp[r, tsl])
                    # k dup to partitions 64:128 (m1 row-packing, odd heads)
                    nc.gpsimd.dma_start(kdup[64:128, tsl], qkv2[0:64, tsl])

            # ---------------- Phase 2: attention ---------------------------
            # v transpose: vT [64, T] -> v t-major [128, 65] per key block
            with tc.tile_pool(name="pst", bufs=2, space="PSUM") as pst:
                for kb in range(NKB):
                    tp = pst.tile([128, 64], F32R)
                    nc.tensor.transpose(
                        tp[:], qkv2[64:128, kb * 128:kb * 128 + 128],
                        ident[64:128, 64:128],
                    )
                    nc.vector.tensor_copy(v_sb[:, kb, 0:64], tp[:])
                    nc.vector.tensor_copy(v_sb[:, kb, 64:65], onecol[:])

            recd = dram.tile([2, 4096], F32)
            with (
                tc.tile_pool(name="probs", bufs=4) as probsp,
                tc.tile_pool(name="small", bufs=1) as smallp,
                tc.tile_pool(name="wp", bufs=8) as wpp,
                tc.tile_pool(name="ymy", bufs=1) as ymyp,
                tc.tile_pool(name="osb", bufs=2) as osbp,
            ):
                y_send = [dram.tile([NCORES, 128, TS], BIG, name=f"ysend{p}",
                                    tag=f"ysend{p}") for p in range(2)]
                # y_my free layout per g: [fh=2, t=256] -> (g*512+fh*256+t)
                y_my = ymyp.tile([128, 16 * TS], BIG)
                ymyv = y_my[:].rearrange("p (g t2) -> p g t2", g=NCORES)
                y_recv = [dram.tile([NCORES, 128, TS], BIG, name=f"yrecv{p}",
                                    tag=f"yrecv{p}") for p in range(2)]
                ps_stack = ExitStack()
                psc = ps_stack.enter_context(
                    tc.tile_pool(name="psc", bufs=3, space="PSUM"))
                psy = ps_stack.enter_context(
                    tc.tile_pool(name="psy", bufs=1, space="PSUM"))
                for pair in range(2):
                    qt_tile = (qkv0, qkv1)[pair]
                    # sumexp reciprocals live on partition 64 (same partition
                    # as the ones-column row of the m2 PSUM output)
                    recs = smallp.tile([128, 8, 512], F32, name="recs", tag="recs")
                    for j in range(NCH):
                        tsl = slice(j * 512, j * 512 + 512)
                        nkb_j = 4 * j + 4
                        yps = [psy.tile([65, 512], F32, name=f"y{h}", tag=f"y{h}")
                               for h in range(2)]
                        for kb0 in range(0, nkb_j, 2):
                            for h in range(2):
                                sc = psc.tile([128, 1024], F32, name="sc", tag="sc")
                                lhs_base = (qkv2[0:64], kdup[64:128])[h]
                                for i, kb in ((0, kb0), (1, kb0 + 1)):
                                    nc.tensor.matmul(
                                        sc[:, i * 512:i * 512 + 512],
                                        lhs_base[:, kb * 128:kb * 128 + 128],
                                        qt_tile[64 * h:64 * h + 64, tsl],
                                        tile_position=(64 * h, 0),
                                    )
                                rag0 = max(0, kb0 - 4 * j) * 128
                                probs = probsp.tile([128, 1024], F32R,
                                                    name="pr", tag="pr")
                                nc.scalar.activation(
                                    probs[:, rag0:1024], sc[:, rag0:1024],
                                    EXP, scale=SCALE)
                                for i, kb in ((0, kb0), (1, kb0 + 1)):
                                    m = kb - 4 * j
                                    if m >= 0:  # diagonal: zero probs above
                                        o = i * 512 + m * 128
                                        # keep kpos<=qt: p<=c; zero where c<p
                                        nc.gpsimd.affine_select(
                                            out=probs[:, o:o + 128],
                                            in_=probs[:, o:o + 128],
                                            compare_op=mybir.AluOpType.is_ge,
                                            fill=0.0, base=0,
                                            pattern=[[1, 128]],
                                            channel_multiplier=-1,
                                        )
                                for i, kb in ((0, kb0), (1, kb0 + 1)):
                                    rag = max(0, (kb - 4 * j)) * 128
                                    nc.tensor.matmul(
                                        yps[h][:, rag:512],
                                        v_sb[:, kb, :],
                                        probs[:, i * 512 + rag:i * 512 + 512],
                                        start=(kb == 0), stop=(kb == nkb_j - 1),
                                    )
                        for h in range(2):
                            hd = pair * 2 + h
                            nc.vector.reciprocal(
                                recs[64:65, h * 4 + j, :], yps[h][64:65, :])
                            nc.vector.tensor_copy(
                                yts[hd][:, tsl], yps[h][0:64, :])
                    # batched normalize for this pair: 2 DMAs + 8 in-place muls
                    nc.gpsimd.dma_start(recd[pair, :], recs[64:65, :, :])
                    recb = smallp.tile([64, 8, 512], BIG, name="recb", tag="recb")
                    rd = recd[pair, :]
                    nc.gpsimd.dma_start(
                        recb[:],
                        bass.AP(tensor=rd.tensor, offset=rd.offset,
                                ap=[[0, 64]] + list(rd.rearrange("(i t) -> i t", i=8).ap)),
                    )
                    for h in range(2):
                        hd = pair * 2 + h
                        for j in range(NCH):
                            tsl = slice(j * 512, j * 512 + 512)
                            nc.vector.tensor_mul(
                                yts[hd][:, tsl], yts[hd][:, tsl],
                                recb[:, h * 4 + j, :])
                    # this pair's AllToAll overlaps the rest of the kernel
                    for h in range(2):
                        hd = pair * 2 + h
                        nc.gpsimd.dma_start(
                            y_send[pair][:, h * 64:h * 64 + 64, :].rearrange(
                                "i f t -> f i t"),
                            yts[hd][:].rearrange("d (i t) -> d i t", i=NCORES),
                        )
                    if for_sim:
                        nc.sync.dma_start(y_recv[pair][:], y_send[pair][:])
                    else:
                        nc.gpsimd.collective_compute(
                            "AllToAll",
                            mybir.AluOpType.bypass,
                            replica_groups=[list(range(NCORES))],
                            ins=[y_send[pair].opt()],
                            outs=[y_recv[pair].opt()],
                        )
                    nc.scalar.dma_start(
                        ymyv[:, :, pair * TS:pair * TS + TS],
                        y_recv[pair][:].rearrange("g p t -> p g t"),
                    )
                ps_stack.close()

                # ---------------- Phase 3: output projection ---------------

                with tc.tile_pool(name="psp", bufs=2, space="PSUM") as psp:
                    wpin = wprojT.ap().rearrange("(g two p) j -> two p g j",
                                                 two=2, p=128)
                    for jc in range(4):
                        jsl = slice(jc * 512, jc * 512 + 512)
                        pp = [psp.tile([128, 512], F32, name=f"pp{tt}", tag=f"pp{tt}")
                              for tt in range(2)]
                        for par in range(2):  # pair-0 features first
                            wp = wpp.tile([128, 8, 512], BIG)
                            nc.sync.dma_start(
                                wp[:], wpin[par:par + 1, :, :, jsl])
                            for g8 in range(8):
                                off = g8 * 512 + par * 256
                                for tt in range(2):
                                    nc.tensor.matmul(
                                        pp[tt][:],
                                        y_my[:, off + tt * 128:off + tt * 128 + 128],
                                        wp[:, g8, :],
                                        start=(par == 0 and g8 == 0),
                                        stop=(par == 1 and g8 == 7),
                                    )
                        for tt in range(2):
                            osbt = osbp.tile([128, 512], F32, name="osbt",
                                             tag="osbt")
                            nc.vector.tensor_add(osbt[:], pp[tt][:],
                                                 bprojb[:, jsl])
                            nc.gpsimd.dma_start(
                                out[tt * 128:tt * 128 + 128, jsl], osbt[:])

    nc.finalize()
    return nc


def _get_nc():
    if "nc" not in _nc_cache:
        _nc_cache["nc"] = _build()
    return _nc_cache["nc"]


def _prepare_in_maps(x, cos, sin, W_attn, b_attn, W_proj, b_proj):
    x = np.asarray(x, dtype=np.float32)
    cos = np.asarray(cos, dtype=np.float32)
    sin = np.asarray(sin, dtype=np.float32)
    W_attn = np.asarray(W_attn, dtype=np.float32)
    b_attn = np.asarray(b_attn, dtype=np.float32)
    W_proj = np.asarray(W_proj, dtype=np.float32)
    b_proj = np.asarray(b_proj, dtype=np.float32)

    big = np.float32
    if USE_BF16:
        import ml_dtypes
        big = ml_dtypes.bfloat16
    xT = np.ascontiguousarray(x[0].T).astype(big)          # [C, T]
    wprojT = np.ascontiguousarray(W_proj.T).astype(big)    # [C(in f), C(out j)]
    bproj = b_proj.reshape(1, C)

    ct, st = cos.T.astype(np.float32), sin.T.astype(np.float32)  # [16, T]
    ropeC = np.ones((128, T), np.float32)
    ropeS = np.zeros((128, T), np.float32)
    for base in (0, 64):
        ropeC[base:base + 16] = ct
        ropeS[base:base + 8] = -st[0:8]
        ropeS[base + 8:base + 16] = st[8:16]

    in_maps = []
    for g in range(NCORES):
        wg = np.ascontiguousarray(W_attn[g * QG:(g + 1) * QG].T).astype(big)
        bg = np.ascontiguousarray(b_attn[g * QG:(g + 1) * QG].reshape(QG, 1))
        in_maps.append({
            "xT": xT, "wqkvT": wg, "bqkv": bg,
            "ropeC": ropeC, "ropeS": ropeS,
            "wprojT": wprojT, "bproj": bproj,
        })
    return in_maps


def kernel(x, cos, sin, W_attn, b_attn, W_proj, b_proj):
    nc = _get_nc()
    in_maps = _prepare_in_maps(x, cos, sin, W_attn, b_attn, W_proj, b_proj)
    res = run_bass_kernel_spmd(nc, in_maps, list(range(NCORES)))
    out = np.concatenate([res.results[g]["out"] for g in range(NCORES)], axis=0)
    return out.reshape(1, T, C).astype(np.float32)

